# revision 68
# baseline (speedup 1.0000x reference)
"""Trainium2 Bass kernel for CollapsedPBFA (collapsed Chebyshev linear attention).

Full-input contract: kernel(x, W_in, W_out) -> (B, S, D) float32.

Sharding: B x H = 2 x 16 = 32 (batch, head) pairs; each of the 8 cores owns
one batch element's 4-head block (cores 0-3 -> b=0, cores 4-7 -> b=1).
QKV projection is column-parallel per head block; the output projection is
row-parallel (each core computes a partial (S, D) product over its 256
hidden columns) and the host sums the per-core partials per batch element.

Structure (v4): the 8 s-tiles are processed as four 2-tile pairs so the
pipeline starts early and flows; tile pools double/triple buffer across
pairs.  Engine assignment balances the four compute engines:
  - PE: QKV matmuls, blocked-triangular cumsum + rank-1 carries, out proj.
  - Scalar (ACT): PSUM evacuations and the square/affine feature ops.
  - Vector (DVE): TT feature ops, row-sum reduces, num tree, den, outh.
  - GpSimd: the bulk Tk*v multiplies and part of Tq*kv.
Out_h transposes ride the DMA engines (dma_start_transpose); the out-proj
PSUM tiles share the QKV pool so the tail double-buffers.

Algebra (unchanged from baseline): beta is nonzero only for T_1..T_5;
beta_p is folded into the per-p triangular constants for the kv cumsum;
the den path runs as a separate 20-channel cumsum (plain tril) with beta
applied via tiny scalar-engine copies; s is block-reversed within each
128-tile so the running prefix of each chunk sits on partition 0 (rank-1
carry matmul).
"""

import sys

for _p in ("/opt/trn_rl_repo", "/root/.axon_site/_ro/trn_rl_repo"):
    if _p not in sys.path:
        sys.path.append(_p)

import numpy as np

import concourse.bacc as bacc
import concourse.bass as bass
import concourse.tile as tile
from concourse import mybir

F32 = mybir.dt.float32
BF16 = mybir.dt.bfloat16

B, S, D = 2, 1024, 1024
H, DH = 16, 64
HPC = 4                    # heads per core
EC = HPC * DH              # 256 feature cols per core side
NP = 5                     # Chebyshev orders 1..5
NS = S // 128              # 8 s-tiles
NKD = D // 128             # 8 k-tiles over d for QKV
EPS_DEN = 1e-7
INV_SQRT_D = 1.0 / 8.0     # 1/sqrt(64)


def _beta():
    j = np.arange(6, dtype=np.float32)
    alpha = (j + 1.0) ** (-1.5)
    tail = np.flip(np.cumsum(np.flip(alpha)))
    beta = np.concatenate([np.zeros(1, np.float32), tail[1:].astype(np.float32),
                           np.zeros(5, np.float32)])
    return beta / beta.sum()          # (11,); nonzero at 1..5


def _bcast(ap, reps):
    """Broadcast a [P, ...] AP by appending a step-0 inner dim of size reps."""
    return bass.AP(tensor=ap.tensor, offset=ap.offset,
                   ap=list(ap.ap) + [[0, reps]])


def _build():
    nc = bacc.Bacc("TRN2", target_bir_lowering=False, debug=False, num_devices=8)

    XT = nc.dram_tensor("xt", [D, S], BF16, kind="ExternalInput")
    WQKVT = nc.dram_tensor("wqkvt", [D, 3 * EC], BF16, kind="ExternalInput")
    WOUTT = nc.dram_tensor("woutt", [EC, D], BF16, kind="ExternalInput")
    LTB = nc.dram_tensor("ltb", [NP, 128, 128], BF16, kind="ExternalInput")
    LTP = nc.dram_tensor("ltp", [128, 128], BF16, kind="ExternalInput")
    PART = nc.dram_tensor("part", [S, D], F32, kind="ExternalOutput")

    AX = mybir.AxisListType.X
    OP = mybir.AluOpType
    ACT = mybir.ActivationFunctionType

    beta = _beta()

    with tile.TileContext(nc) as tc:
        with (
            nc.allow_low_precision(reason="bf16 feature pipeline by design"),
            tc.tile_pool(name="persist", bufs=1) as pp,
            tc.tile_pool(name="work", bufs=2) as wp,
            tc.tile_pool(name="ps_qkv", bufs=2, space="PSUM") as ps_qkv,
            tc.tile_pool(name="ps_kv", bufs=2, space="PSUM") as ps_kv,
            tc.tile_pool(name="ps_o", bufs=2, space="PSUM") as ps_o,
        ):
            xt = pp.tile([128, NKD, S], BF16)
            wqkvt = pp.tile([128, NKD, 3 * EC], BF16)
            woutt = pp.tile([128, 2, D], BF16)
            ltb = pp.tile([128, NP, 128], BF16)
            ltp = pp.tile([128, 128], BF16)
            ones1 = pp.tile([1, 128], BF16)
            outt = pp.tile([128, 2, S], BF16)

            # weights first, then x column-halves: tile-0's QKV inputs land early
            for k in range(NKD):
                nc.sync.dma_start(out=wqkvt[:, k, :], in_=WQKVT[128 * k:128 * (k + 1), :])
            for sh in range(2):
                ss = slice(512 * sh, 512 * (sh + 1))
                for k in range(NKD):
                    nc.sync.dma_start(out=xt[:, k, ss],
                                      in_=XT[128 * k:128 * (k + 1), ss])
            for k in range(2):
                nc.sync.dma_start(out=woutt[:, k, :], in_=WOUTT[128 * k:128 * (k + 1), :])
            for p in range(NP):
                nc.sync.dma_start(out=ltb[:, p, :], in_=LTB[p])
            nc.sync.dma_start(out=ltp, in_=LTP.ap())
            nc.vector.memset(ones1, 1.0)

            kvt_prev = None
            wprev = 1
            for ii in ((0, 1), (2, 3), (4, 5), (6, 7)):
                W = len(ii)
                # t rows per p: [q 0:256 | k 256:512 | v 512:768] (v on p=0)
                t = wp.tile([128, W, NP, 768], BF16, tag="t", bufs=3,
                            padded_shape=[128, 2, NP, 768])

                # ---------------- QKV + evac (one 768-wide copy per tile)
                for jj, i in enumerate(ii):
                    si = slice(128 * i, 128 * (i + 1))
                    qkv = ps_qkv.tile([128, 768], F32, tag="qkv")
                    for k in range(NKD):
                        lhs = xt[:, k, si]
                        nc.tensor.matmul(qkv[:, 0:512], lhs, wqkvt[:, k, 0:512],
                                         start=(k == 0), stop=(k == NKD - 1))
                        nc.tensor.matmul(qkv[:, 512:768], lhs, wqkvt[:, k, 512:768],
                                         start=(k == 0), stop=(k == NKD - 1))
                    with tc.high_priority():
                        nc.scalar.copy(out=t[:, jj, 0, 0:512], in_=qkv[:, 0:512])
                    nc.scalar.copy(out=t[:, jj, 0, 512:768],
                                   in_=qkv[:, 512:768])

                # ---------------- Chebyshev features, wide over the pair
                x1 = t[:, :, 0, 0:512]
                t2 = t[:, :, 1, 0:512]
                t3 = t[:, :, 2, 0:512]
                t4 = t[:, :, 3, 0:512]
                t5 = t[:, :, 4, 0:512]
                m2 = wp.tile([128, W, 512], BF16, tag="sq", padded_shape=[128, 2, 512])
                nc.scalar.activation(out=m2, in_=x1, func=ACT.Square)
                nc.scalar.activation(out=t2, in_=m2, func=ACT.Copy,
                                     scale=2.0, bias=-1.0)
                w3 = wp.tile([128, W, 512], BF16, tag="tt", bufs=3, padded_shape=[128, 2, 512])
                nc.vector.tensor_scalar(out=w3, in0=t2, scalar1=4.0, scalar2=-2.0,
                                        op0=OP.mult, op1=OP.add)
                nc.vector.tensor_tensor(out=t3, in0=x1, in1=w3, op=OP.mult)
                m4 = wp.tile([128, W, 512], BF16, tag="sq", padded_shape=[128, 2, 512])
                nc.scalar.activation(out=m4, in_=t2, func=ACT.Square)
                nc.scalar.activation(out=t4, in_=m4, func=ACT.Copy,
                                     scale=2.0, bias=-1.0)
                m5 = wp.tile([128, W, 512], BF16, tag="tt", bufs=3, padded_shape=[128, 2, 512])
                nc.gpsimd.tensor_mul(m5, t2, t3)
                nc.vector.tensor_tensor(out=t5, in0=m5, in1=x1, op=OP.subtract)

                # ---------------- row-sums (per head, q then k) for den
                qsks = wp.tile([128, W, NP, 2 * HPC], BF16, tag="qsks", padded_shape=[128, 2, NP, 2 * HPC])
                nc.vector.tensor_reduce(
                    out=qsks,
                    in_=t[:, :, :, 0:512].rearrange("a j p (h d) -> a j p h d",
                                                    h=2 * HPC),
                    axis=AX, op=OP.add)
                qs = qsks[:, :, :, 0:HPC]
                ks = qsks[:, :, :, HPC:2 * HPC]

                # ---------------- Tv = Tk * v  (gp: p0-2, vector: p3-4)
                tv = wp.tile([128, W, NP, 256], BF16, tag="tv", padded_shape=[128, 2, NP, 256])
                vsl = t[:, :, 0, 512:768]
                for p in range(NP):
                    eng = nc.gpsimd if p < 3 else nc.vector
                    eng.tensor_mul(tv[:, :, p, :], t[:, :, p, 256:512], vsl)

                # ---------------- causal cumsum: 3 PSUM-packed chains per tile
                # kvt rows: p0..4 -> [kv 0:256]; p4 row also holds ks at 256:276
                kvt = wp.tile([128, W, NP, 276], BF16, tag="kvt", bufs=3, padded_shape=[128, 2, NP, 276])
                for jj, i in enumerate(ii):
                    first = (i == 0)
                    hp = tc.high_priority()
                    hp.__enter__()

                    def carry_row(p, lo, hi):
                        return (kvt[0:1, jj - 1, p, lo:hi] if jj
                                else kvt_prev[0:1, wprev - 1, p, lo:hi])

                    kva = ps_kv.tile([128, 512], F32, tag="kv")
                    for p in (0, 1):
                        o = kva[:, 256 * p:256 * (p + 1)]
                        nc.tensor.matmul(o, ltb[:, p, :], tv[:, jj, p, :],
                                         start=True, stop=first)
                        if not first:
                            nc.tensor.matmul(o, ones1, carry_row(p, 0, 256),
                                             start=False, stop=True)
                    nc.scalar.copy(out=kvt[:, jj, 0:2, 0:256],
                                   in_=kva.rearrange("a (p d) -> a p d", p=2))
                    kvb = ps_kv.tile([128, 512], F32, tag="kv")
                    for p in (2, 3):
                        o = kvb[:, 256 * (p - 2):256 * (p - 1)]
                        nc.tensor.matmul(o, ltb[:, p, :], tv[:, jj, p, :],
                                         start=True, stop=first)
                        if not first:
                            nc.tensor.matmul(o, ones1, carry_row(p, 0, 256),
                                             start=False, stop=True)
                    nc.scalar.copy(out=kvt[:, jj, 2:4, 0:256],
                                   in_=kvb.rearrange("a (p d) -> a p d", p=2))
                    kvc = ps_kv.tile([128, 512], F32, tag="kv")
                    nc.tensor.matmul(kvc[:, 0:256], ltb[:, 4, :], tv[:, jj, 4, :],
                                     start=True, stop=first)
                    if not first:
                        nc.tensor.matmul(kvc[:, 0:256], ones1,
                                         carry_row(4, 0, 256),
                                         start=False, stop=True)
                    nc.scalar.copy(out=kvt[:, jj, 4, 0:256], in_=kvc[:, 0:256])
                    # ks cumsum decoupled: only this tiny part waits on the
                    # row-sum reduce
                    nc.tensor.matmul(kvc[:, 256:276], ltp, ks[:, jj],
                                     start=True, stop=first)
                    if not first:
                        nc.tensor.matmul(kvc[:, 256:276], ones1,
                                         carry_row(4, 256, 276),
                                         start=False, stop=True)
                    nc.scalar.copy(out=kvt[:, jj, 4, 256:276],
                                   in_=kvc[:, 256:276])
                    hp.__exit__(None, None, None)
                kvt_prev = kvt
                wprev = W

                # ---------------- num: prods = Tq_p * kv_p (beta in LTB), tree
                prods = wp.tile([128, W, NP, 256], BF16, tag="prods", padded_shape=[128, 2, NP, 256])
                nc.gpsimd.tensor_mul(prods[:, :, 0:4, :], t[:, :, 0:4, 0:256],
                                     kvt[:, :, 0:4, 0:256])
                nc.vector.tensor_mul(prods[:, :, 4, :], t[:, :, 4, 0:256],
                                     kvt[:, :, 4, 0:256])
                a01 = wp.tile([128, W, 256], BF16, tag="a01",
                              padded_shape=[128, 2, 256])
                a23 = wp.tile([128, W, 256], BF16, tag="a23",
                              padded_shape=[128, 2, 256])
                numq = wp.tile([128, W, 256], BF16, tag="numq",
                               padded_shape=[128, 2, 256])
                nc.vector.tensor_add(a01, prods[:, :, 0, :], prods[:, :, 1, :])
                nc.vector.tensor_add(a23, prods[:, :, 2, :], prods[:, :, 3, :])
                nc.vector.tensor_add(a01, a01, prods[:, :, 4, :])
                nc.vector.tensor_tensor(out=numq, in0=a01, in1=a23, op=OP.add)

                # ---------------- den: bqs = beta_p*qs_p (scalar), TT + reduce
                ksC = kvt[:, :, 4, 256:276].rearrange("a j (p h) -> a j p h",
                                                      p=NP)
                bqs = wp.tile([128, W, NP, HPC], BF16, tag="bqs", padded_shape=[128, 2, NP, HPC])
                with tc.high_priority(offset=-100000):
                    for p in range(NP):
                        nc.vector.tensor_scalar_mul(out=bqs[:, :, p, :],
                                                    in0=qs[:, :, p, :],
                                                    scalar1=float(beta[p + 1]) * (0.25 if p == 2 else 1.0))
                dpr = wp.tile([128, W, NP, HPC], F32, tag="dpr", padded_shape=[128, 2, NP, HPC])
                nc.vector.tensor_tensor(out=dpr, in0=bqs, in1=ksC, op=OP.mult)
                den4 = wp.tile([128, W, HPC], F32, tag="den4", padded_shape=[128, 2, HPC])
                rden = wp.tile([128, W, HPC], BF16, tag="rden", padded_shape=[128, 2, HPC])
                nc.vector.tensor_reduce(out=den4,
                                        in_=dpr.rearrange("a j p h -> a j h p"),
                                        axis=AX, op=OP.add)
                nc.vector.tensor_scalar_add(out=den4, in0=den4, scalar1=EPS_DEN)
                nc.vector.reciprocal(out=rden, in_=den4)
                outh = wp.tile([128, W, 256], BF16, tag="outh",
                               padded_shape=[128, 2, 256])
                nc.vector.tensor_tensor(
                    out=outh.rearrange("a j (h d) -> a j h d", h=HPC),
                    in0=numq.rearrange("a j (h d) -> a j h d", h=HPC),
                    in1=_bcast(rden, DH), op=OP.mult)

                # ---------------- out_h transpose (DMA xbar) + out proj
                for jj, i in enumerate(ii):
                    si = slice(128 * i, 128 * (i + 1))
                    nc.sync.dma_start_transpose(out=outt[:, :, si],
                                                in_=outh[:, jj, :])
                    outfull = wp.tile([128, D], F32, tag="outfull")
                    for n in range(2):
                        op_ps = ps_o.tile([128, 512], F32, tag="op")
                        for kt in range(2):
                            nc.tensor.matmul(op_ps, outt[:, kt, si],
                                             woutt[:, kt, 512 * n:512 * (n + 1)],
                                             start=(kt == 0), stop=(kt == 1))
                        with tc.high_priority(offset=-100000):
                            nc.scalar.copy(out=outfull[:, 512 * n:512 * (n + 1)],
                                           in_=op_ps)
                    nc.sync.dma_start(out=PART[si, :], in_=outfull)

    nc.compile()
    return nc


_NC = None


def _get_nc():
    global _NC
    if _NC is None:
        _NC = _build()
    return _NC


def _stage_inputs(x, W_in, W_out):
    import ml_dtypes
    bf = ml_dtypes.bfloat16
    beta = _beta()
    tri = np.tril(np.ones((128, 128), np.float32))
    ltb = np.stack([beta[p] * (0.25 if p == 3 else 1.0) * tri
                    for p in range(1, 6)]).astype(bf)
    ltp = tri.astype(bf)
    in_maps = []
    for c in range(8):
        b, hb = divmod(c, 4)
        rs = slice(256 * hb, 256 * (hb + 1))
        wq = W_in[0 * D + 256 * hb:0 * D + 256 * (hb + 1)] * INV_SQRT_D
        wk = W_in[1 * D + 256 * hb:1 * D + 256 * (hb + 1)] * INV_SQRT_D
        wv = W_in[2 * D + 256 * hb:2 * D + 256 * (hb + 1)]
        wqkvt = np.ascontiguousarray(
            np.concatenate([wq, wk, wv], axis=0).T).astype(bf)
        xrev = x[b].T.reshape(D, NS, 128)[:, :, ::-1].reshape(D, S)
        in_maps.append({
            "xt": np.ascontiguousarray(xrev).astype(bf),
            "wqkvt": wqkvt,
            "woutt": np.ascontiguousarray(W_out[:, rs].T).astype(bf),
            "ltb": ltb,
            "ltp": ltp,
        })
    return in_maps


def kernel(x, W_in, W_out):
    from concourse.bass_utils import run_bass_kernel_spmd

    x = np.asarray(x, dtype=np.float32)
    W_in = np.asarray(W_in, dtype=np.float32)
    W_out = np.asarray(W_out, dtype=np.float32)
    nc = _get_nc()
    in_maps = _stage_inputs(x, W_in, W_out)
    res = run_bass_kernel_spmd(nc, in_maps, core_ids=list(range(8)))
    out = np.zeros((B, S, D), dtype=np.float32)
    for c in range(8):
        part = res.results[c]["part"].reshape(NS, 128, D)[:, ::-1, :].reshape(S, D)
        out[c // 4] += part
    return out
